# revision 10
# baseline (speedup 1.0000x reference)
"""GravityGAE Trainium2 Bass kernel.

Model: h = relu(adj @ (x @ W1)); z = adj @ (h @ W2)
       dist = x1 - 2 z'z'^T + x1^T + eps ; out = mass^T - log(dist)

Sharding: row-shard x/adj/output over 8 cores; replicate W1/W2; AllGather the
small intermediates (p = x@W1, q = h@W2, decode vectors).

Each core computes its [1024, 8192] slab of the output, stored TRANSPOSED
(j on partitions) so the per-column mass term lands on partitions; the host
transposes each slab back when assembling the full output.

All matmul contractions run on the PE partition dim, so adj tiles are
transposed on-chip (PE identity transpose -> PSUM -> SBUF copy). The first
few transposed chunks stay resident in SBUF and are reused in pass 2 to cut
the second HBM read of adj.
"""

import numpy as np

import concourse.bass as bass
import concourse.mybir as mybir
import concourse.tile as tile
from concourse import bacc
from concourse import bass_utils
from concourse.masks import make_identity

F32 = mybir.dt.float32
N = 8192
NCORES = 8
SLAB = N // NCORES          # 1024 rows per core
NCHUNK = SLAB // 128        # 8 i-chunks of 128 rows
KT = N // 128               # 64 k-tiles
IN_DIM = 512
HID = 64
ZD = 33                     # 32 latent + 1 mass
EPS = 0.01

# how many (ci, half) units of transposed adj stay resident for pass 2.
# each unit is [128k x 32t x 128i] -> 2MB; 16 units total.
CACHE_UNITS = 2

_nc_cache = {}


def _build():
    if "nc" in _nc_cache:
        return _nc_cache["nc"]
    nc = bacc.Bacc("TRN2", target_bir_lowering=False, debug=False,
                   enable_asserts=False, num_devices=NCORES)
    x_in = nc.dram_tensor("x_slab", [SLAB, IN_DIM], F32, kind="ExternalInput")
    adj_in = nc.dram_tensor("adj_slab", [SLAB, N], F32, kind="ExternalInput")
    w1_in = nc.dram_tensor("w1", [IN_DIM, HID], F32, kind="ExternalInput")
    w2_in = nc.dram_tensor("w2", [HID, ZD], F32, kind="ExternalInput")
    outT = nc.dram_tensor("outT", [N, SLAB], F32, kind="ExternalOutput")

    # adj viewed as [ci, pi, t, kf] for k-block slicing
    adj_v = adj_in.ap().rearrange("(ci pi) (t kf) -> pi ci t kf", pi=128, kf=128)
    rg = [list(range(NCORES))]

    with tile.TileContext(nc) as tc:
        with tc.tile_pool(name="const", bufs=1) as constp, \
             tc.tile_pool(name="stage", bufs=2) as stagep, \
             tc.tile_pool(name="cacheT", bufs=CACHE_UNITS) as cachep, \
             tc.tile_pool(name="transT", bufs=2) as transp, \
             tc.tile_pool(name="small", bufs=1) as smallp, \
             tc.tile_pool(name="big", bufs=1) as bigp, \
             tc.tile_pool(name="outst", bufs=4) as outp, \
             tc.tile_pool(name="ptr", bufs=2, space="PSUM") as ptr, \
             tc.tile_pool(name="pacc", bufs=1, space="PSUM") as pacc, \
             tc.tile_pool(name="psml", bufs=1, space="PSUM") as psml, \
             tc.tile_pool(name="pdec", bufs=2, space="PSUM") as pdec, \
             tc.tile_pool(name="dram", bufs=1, space="DRAM") as dramp:

            ident = constp.tile([128, 128], F32)
            make_identity(nc, ident[:])
            ones32 = constp.tile([32, 1], F32)
            nc.vector.memset(ones32[:], 1.0)
            ones11 = constp.tile([1, 1], F32)
            nc.vector.memset(ones11[:], 1.0)
            ones_row = constp.tile([1, SLAB], F32)
            nc.vector.memset(ones_row[:], 1.0)

            # ---- weights in ----
            w1_sb = constp.tile([128, 4, HID], F32)
            nc.sync.dma_start(
                w1_sb[:], w1_in.ap().rearrange("(ft p) h -> p ft h", p=128))
            w2_sb = constp.tile([HID, ZD], F32)
            nc.sync.dma_start(w2_sb[:], w2_in.ap())

            # ---- pass 0: p_local = x_slab @ W1, AllGather -> p_full ----
            p_loc = bigp.tile([128, NCHUNK, HID], F32)  # [p, ci, h]
            for ci in range(NCHUNK):
                xc = stagep.tile([128, IN_DIM], F32, tag="xc")
                nc.sync.dma_start(xc[:], x_in.ap()[ci * 128:(ci + 1) * 128, :])
                pp = psml.tile([128, HID], F32, tag="pq")
                for ft in range(4):
                    xT_ps = ptr.tile([128, 128], F32, tag="tr")
                    nc.tensor.transpose(
                        xT_ps[:], xc[:, ft * 128:(ft + 1) * 128], ident[:])
                    xT_sb = outp.tile([128, 128], F32, tag="lsb")
                    nc.vector.tensor_copy(xT_sb[:], xT_ps[:])
                    nc.tensor.matmul(pp[:], xT_sb[:], w1_sb[:, ft, :],
                                     start=(ft == 0), stop=(ft == 3),
                                     skip_group_check=True)
                nc.vector.tensor_copy(p_loc[:, ci, :], pp[:])
            p_bnc = dramp.tile([SLAB, HID], F32)
            nc.sync.dma_start(
                p_bnc[:].rearrange("(ci p) h -> p ci h", p=128), p_loc[:])
            p_ag = nc.dram_tensor("p_ag", [N, HID], F32, kind="Internal",
                                  addr_space="Shared")
            nc.gpsimd.collective_compute(
                "AllGather", mybir.AluOpType.bypass, replica_groups=rg,
                ins=[p_bnc.opt()], outs=[p_ag.ap()])
            p_full = bigp.tile([128, KT, HID], F32)  # k-tile t: [:, t, :]
            nc.sync.dma_start(
                p_full[:], p_ag.ap().rearrange("(t p) h -> p t h", p=128))

            # ---- pass 1: hT = relu(adj @ p)^T  [64, 1024] ----
            hT_ps = pacc.tile([HID, SLAB], F32, tag="acc")
            cache_tiles = {}
            for ci in range(NCHUNK):
                for hf in range(2):  # halves of the k range
                    unit = ci * 2 + hf
                    if unit < CACHE_UNITS:
                        aT = cachep.tile([128, 32, 128], F32, tag="cache")
                        cache_tiles[unit] = aT
                    else:
                        aT = transp.tile([128, 32, 128], F32, tag="trans")
                    nat = stagep.tile([128, 32, 128], F32, tag="nat")
                    # load 32 k-blocks for chunk ci: [128, 32, 128]
                    nc.sync.dma_start(
                        nat[:], adj_v[:, ci, hf * 32:(hf + 1) * 32, :])
                    for tt in range(32):
                        t = hf * 32 + tt
                        tp = ptr.tile([128, 128], F32, tag="tr")
                        nc.tensor.transpose(tp[:], nat[:, tt, :], ident[:])
                        nc.vector.tensor_copy(aT[:, tt, :], tp[:])
                        nc.tensor.matmul(
                            hT_ps[:, ci * 128:(ci + 1) * 128],
                            p_full[:, t, :], aT[:, tt, :],
                            start=(t == 0), stop=(t == KT - 1),
                            skip_group_check=True)
            hT = bigp.tile([HID, SLAB], F32)
            nc.scalar.activation(hT[:], hT_ps[:],
                                 mybir.ActivationFunctionType.Relu)

            # ---- q_local = h @ W2 ; AllGather -> q_full ----
            q_loc = bigp.tile([128, NCHUNK, ZD], F32)
            for ci in range(NCHUNK):
                qp = psml.tile([128, ZD], F32, tag="pq")
                nc.tensor.matmul(qp[:], hT[:, ci * 128:(ci + 1) * 128],
                                 w2_sb[:], start=True, stop=True,
                                 skip_group_check=True)
                nc.vector.tensor_copy(q_loc[:, ci, :], qp[:])
            q_bnc = dramp.tile([SLAB, ZD], F32)
            nc.sync.dma_start(
                q_bnc[:].rearrange("(ci p) h -> p ci h", p=128), q_loc[:])
            q_ag = nc.dram_tensor("q_ag", [N, ZD], F32, kind="Internal",
                                  addr_space="Shared")
            nc.gpsimd.collective_compute(
                "AllGather", mybir.AluOpType.bypass, replica_groups=rg,
                ins=[q_bnc.opt()], outs=[q_ag.ap()])
            q_full = bigp.tile([128, KT, ZD], F32)
            nc.sync.dma_start(
                q_full[:], q_ag.ap().rearrange("(t p) h -> p t h", p=128))

            # ---- pass 2: zT = (adj @ q)^T  [33, 1024] ----
            zT_ps = pacc.tile([ZD, SLAB], F32, tag="acc")
            for ci in range(NCHUNK):
                for hf in range(2):
                    unit = ci * 2 + hf
                    if unit in cache_tiles:
                        aT = cache_tiles[unit]
                    else:
                        aT = transp.tile([128, 32, 128], F32, tag="trans")
                        nat = stagep.tile([128, 32, 128], F32, tag="nat")
                        nc.sync.dma_start(
                            nat[:], adj_v[:, ci, hf * 32:(hf + 1) * 32, :])
                        for tt in range(32):
                            tp = ptr.tile([128, 128], F32, tag="tr")
                            nc.tensor.transpose(tp[:], nat[:, tt, :], ident[:])
                            nc.vector.tensor_copy(aT[:, tt, :], tp[:])
                    for tt in range(32):
                        t = hf * 32 + tt
                        nc.tensor.matmul(
                            zT_ps[:, ci * 128:(ci + 1) * 128],
                            q_full[:, t, :], aT[:, tt, :],
                            start=(t == 0), stop=(t == KT - 1),
                            skip_group_check=True)
            zT = bigp.tile([ZD, SLAB], F32)
            nc.vector.tensor_copy(zT[:], zT_ps[:])

            # ---- decode vector assembly ----
            # x1 = sum_f z_f^2 (latent only), as a row [1, 1024]
            z2 = smallp.tile([32, SLAB], F32, tag="z2")
            nc.scalar.activation(z2[:], zT[0:32, :],
                                 mybir.ActivationFunctionType.Square)
            x1row = smallp.tile([1, SLAB], F32, tag="x1row")
            for half in range(2):
                x1_ps = psml.tile([1, 512], F32, tag="x1")
                nc.tensor.matmul(x1_ps[:],
                                 ones32[:],
                                 z2[:, half * 512:(half + 1) * 512],
                                 start=True, stop=True,
                                 skip_group_check=True)
                nc.vector.tensor_copy(x1row[:, half * 512:(half + 1) * 512],
                                      x1_ps[:])
            # b_local rows: [0:32] = -2*z ; [32] = 1 ; [33] = x1 ; [34] = mass
            b_loc = smallp.tile([35, SLAB], F32, tag="bloc")
            nc.vector.tensor_scalar_mul(b_loc[0:32, :], zT[0:32, :], -2.0)
            nc.vector.memset(b_loc[32:33, :], 1.0)
            nc.sync.dma_start(b_loc[33:34, :], x1row[:])
            nc.sync.dma_start(b_loc[34:35, :], zT[32:33, :])
            # a_local rows: [0:32] = z ; [32] = x1 + eps ; [33] = 1
            a_loc = smallp.tile([34, SLAB], F32, tag="aloc")
            nc.vector.tensor_copy(a_loc[0:32, :], zT[0:32, :])
            nc.vector.tensor_scalar_add(a_loc[32:33, :], x1row[:], EPS)
            nc.sync.dma_start(a_loc[33:34, :], ones_row[:])

            b_bnc = dramp.tile([35, SLAB], F32)
            nc.sync.dma_start(b_bnc[:], b_loc[:])
            b_ag = nc.dram_tensor("b_ag", [NCORES, 35, SLAB], F32,
                                  kind="Internal", addr_space="Shared")
            nc.gpsimd.collective_compute(
                "AllGather", mybir.AluOpType.bypass, replica_groups=rg,
                ins=[b_bnc.opt()], outs=[b_ag.ap()])

            # mass column per j-tile: transpose [1,128] -> [128,1]
            massrow = constp.tile([1, NCORES, SLAB], F32)
            nc.sync.dma_start(
                massrow[:], b_ag.ap()[:, 34:35, :].rearrange("r o i -> o r i"))
            masscol = constp.tile([128, KT], F32)
            for J in range(KT):
                mp = ptr.tile([128, 1], F32, tag="tr")
                nc.tensor.matmul(mp[:],
                                 massrow[:, J // NCHUNK,
                                         (J % NCHUNK) * 128:(J % NCHUNK) * 128 + 128],
                                 ones11[:], start=True, stop=True,
                                 skip_group_check=True)
                nc.vector.tensor_copy(masscol[:, J:J + 1], mp[:])

            # ---- decode: outT[jtile, iblk] ----
            for J in range(KT):
                r, lo = J // NCHUNK, (J % NCHUNK) * 128
                btile = outp.tile([34, 128], F32, tag="bt")
                nc.sync.dma_start(btile[:], b_ag.ap()[r, 0:34, lo:lo + 128])
                bt = btile[:]
                for ib in range(2):
                    dps = pdec.tile([128, 512], F32, tag="dec")
                    nc.tensor.matmul(dps[:], bt,
                                     a_loc[:, ib * 512:(ib + 1) * 512],
                                     start=True, stop=True,
                                     skip_group_check=True)
                    lsb = outp.tile([128, 512], F32, tag="lsb")
                    nc.scalar.activation(lsb[:], dps[:],
                                         mybir.ActivationFunctionType.Ln)
                    osb = outp.tile([128, 512], F32, tag="osb")
                    nc.gpsimd.tensor_scalar(
                        osb[:], lsb[:], masscol[:, J:J + 1], -1.0,
                        op0=mybir.AluOpType.subtract,
                        op1=mybir.AluOpType.mult)
                    nc.sync.dma_start(
                        outT.ap()[J * 128:(J + 1) * 128,
                                  ib * 512:(ib + 1) * 512],
                        osb[:])
    nc.finalize()
    _nc_cache["nc"] = nc
    return nc


def kernel(x, adj, W1, W2):
    nc = _build()
    in_maps = []
    for c in range(NCORES):
        sl = slice(c * SLAB, (c + 1) * SLAB)
        in_maps.append({
            "x_slab": np.ascontiguousarray(x[sl]),
            "adj_slab": np.ascontiguousarray(adj[sl]),
            "w1": W1, "w2": W2,
        })
    res = bass_utils.run_bass_kernel_spmd(
        nc, in_maps, core_ids=list(range(NCORES)))
    out = np.empty((N, N), dtype=np.float32)
    for c in range(NCORES):
        out[c * SLAB:(c + 1) * SLAB, :] = res.results[c]["outT"].T
    return out
